# revision 89
# baseline (speedup 1.0000x reference)
"""Trainium2 Bass kernel for nn_EntailmentSelfAttention.

Strategy
--------
Data-parallel over batch n: 16 batches -> 8 cores x 2 batches.

Per (n, head h, sub-problem s) the reference computes
    energy[q,k] = (q_raw @ Wq^T) @ (k_raw @ Wk^T)^T  (+ -1e20 on masked q rows)
    att = softmax(energy / 32, axis=q)               (softmax over the QUERY axis)
    out[q,d] = att[q,:] @ (v_raw @ Wv^T)
followed by torch-reshape channel shuffles and a final x @ Wo^T + bo.

Weight-only algebra folded on the host (all constants):
    A      = Wq^T @ Wk  so  energy = q_raw @ A @ k_raw^T   (removes 2 projections)
    Wv     is folded into a permuted Wo (Wo3), absorbing both torch-reshape
           channel interleavings, so the attention output (pre-Wv) feeds the
           output projection directly.
The -1e20 query mask rides the contraction as an extra row: host appends a
mask row to q^T and a ones row to k^T / A so the energy matmul itself adds
the mask.  Softmax skips max-subtraction (logits are bounded |e/32| < ~1;
masked entries give exp(-3e18)=0 exactly).

All matmul operands are bf16 (psum accumulation stays f32): same PE rate as
f32r (1 cycle/row) but half the DMA traffic and SBUF footprint.  Logits are
small so bf16 costs ~1% L2 error total, well inside the 2e-2 gate.

Schedule: a software-pipelined slot loop.  Unit u=(group g, head j); slot u
issues on PE in order: B_u, attnxv_(u-3), energy_(u-1), and 4 of the previous
group's 32 output-projection matmuls.  The 3-slot attnxv skew gives the
serial softmax chain (exp -> pair-sum -> q-sum -> recip -> v-scale) two full
slots of slack.  Per-head work is merged into single wide instructions
(one [128,1024] exp, one q-sum, one recip; attn-out copies pair two heads)
to amortize the fixed SBUF/PSUM access latencies.  Engine assignment keeps
PE the clean bottleneck (PE ~62us busy, Act ~53, DVE ~53, Pool ~26,
DMA ~31 of a ~81us span):
  Act : exp(e/32), attn-out psum->sbuf copies, B copies (odd heads)
  DVE : pair-sum + q-sum + reciprocal, B copies (even heads), bias add
  Pool: v-row scaling only (sbuf-only engine; no SWDGE DMAs - they are slow)
  SP  : every DMA (HWDGE); per-group stores merged into two [128,1024] rows
PSUM: B 1 bank | energy 2x2 banks (double-buffered, the critical decoupling)
| attnxv 1 | projection 2.
"""

import os
import sys

sys.path.insert(0, "/opt/trn_rl_repo")

from contextlib import ExitStack

import numpy as np

import concourse.bass as bass
import concourse.mybir as mybir
import concourse.tile as tile
from concourse.bass_utils import run_bass_kernel_spmd

# -------------------------------------------------------------------------
# problem constants (hardcoded per contract)
N, L, E, S = 16, 256, 1024, 2
H, D = 16, 64
NC = 8                      # cores
NB = N // NC                # batches per core
NG = NB * S                 # pipeline groups per core: (nb, head-group) pairs
NU = NG * 8                 # pipeline units: (group, head-in-group)
SCALE = 1.0 / 32.0          # 1/sqrt(E)
F32 = mybir.dt.float32
BF16 = mybir.dt.bfloat16


def _bcast_rows(ap, parts):
    """Broadcast a [1, ...] SBUF AP across `parts` partitions (step 0)."""
    return bass.AP(tensor=ap.tensor, offset=ap.offset,
                   ap=[[0, parts]] + list(ap.ap[1:]))


def fix_multi_waits(nc):
    """This walrus build accepts only ONE sem wait per instruction: split any
    instruction carrying more into preceding same-engine NOPs."""
    ctr = 0
    for f in nc.m.functions:
        for bb in f.blocks:
            insts = bb.instructions
            out = []
            changed = False
            for inst in insts:
                si = inst.sync_info
                if si is not None and si.on_wait and len(si.on_wait) > 1:
                    waits = list(si.on_wait)
                    for w in waits[:-1]:
                        ctr += 1
                        nop = mybir.InstNoOp(
                            name=f"I-waitsplit-{ctr}", ins=[], outs=[])
                        nop.engine = inst.engine
                        nop.sync_info = mybir.SyncInfo(on_wait=[w], on_update=[])
                        out.append(nop)
                    inst.sync_info = mybir.SyncInfo(
                        on_wait=[waits[-1]], on_update=list(si.on_update or []))
                    changed = True
                out.append(inst)
            if changed:
                bb.instructions = out


def build_program(fix_waits=True):
    nc = bass.Bass()
    qte = nc.dram_tensor("qte", [NB, D + 1, H, S, L], BF16, kind="ExternalInput").ap()
    kte = nc.dram_tensor("kte", [NB, D + 1, H, S, L], BF16, kind="ExternalInput").ap()
    vh = nc.dram_tensor("vh", [NB, 128, H, S, 2, D], BF16, kind="ExternalInput").ap()
    ast = nc.dram_tensor("ast", [D + 1, D + 1], BF16, kind="ExternalInput").ap()
    wo3t = nc.dram_tensor("wo3t", [128, 8, E], BF16, kind="ExternalInput").ap()
    bo = nc.dram_tensor("bo", [1, E], F32, kind="ExternalInput").ap()
    out = nc.dram_tensor("out", [NB, L, S, E], BF16, kind="ExternalOutput").ap()

    with tile.TileContext(nc) as tc, ExitStack() as ctx:
        singles = ctx.enter_context(tc.tile_pool(name="singles", bufs=1))
        gq = ctx.enter_context(tc.tile_pool(name="gq", bufs=2))
        gk = ctx.enter_context(tc.tile_pool(name="gk", bufs=2))
        gv = ctx.enter_context(tc.tile_pool(name="gv", bufs=2))
        bsb = ctx.enter_context(tc.tile_pool(name="bsb", bufs=6))
        attp = ctx.enter_context(tc.tile_pool(name="attp", bufs=6))
        ssp = ctx.enter_context(tc.tile_pool(name="ssp", bufs=6))
        rrp = ctx.enter_context(tc.tile_pool(name="rrp", bufs=6))
        hp = ctx.enter_context(tc.tile_pool(name="hp", bufs=4))
        vsp0 = ctx.enter_context(tc.tile_pool(name="vsp0", bufs=6))
        vsp1 = ctx.enter_context(tc.tile_pool(name="vsp1", bufs=6))
        atp = ctx.enter_context(tc.tile_pool(name="atp", bufs=10))
        osb = ctx.enter_context(tc.tile_pool(name="osb", bufs=6))
        bps = ctx.enter_context(tc.tile_pool(name="bps", bufs=1, space="PSUM"))
        eps = ctx.enter_context(tc.tile_pool(name="eps", bufs=2, space="PSUM"))
        tps = ctx.enter_context(tc.tile_pool(name="tps", bufs=1, space="PSUM"))
        fps = ctx.enter_context(tc.tile_pool(name="fps", bufs=2, space="PSUM"))

        ast_sb = singles.tile([D + 1, D + 1], BF16)
        wo_sb = singles.tile([128, 8, E], BF16)
        bo_sb = singles.tile([128, E], F32)

        # vs tiles are [128,128] stationaries with a persistent zero half so
        # the s=1 attnxv matmul lands on psum partitions 64..127.  Zero the
        # halves once per pool slot (memset); later generations only write
        # the live v half.
        for pool, tg, zsl in ((vsp0, "vs0", bass.ts(1, 64)),
                              (vsp1, "vs1", bass.ts(0, 64))):
            for _ in range(6):
                t = pool.tile([128, 2 * D], BF16, tag=tg)
                nc.gpsimd.memset(t[:, zsl], 0.0)

        def load_group(g):
            nb, hg = divmod(g, S)
            h0 = 8 * hg
            kt_g = gk.tile([D + 1, 8, S, L], BF16)
            qt_g = gq.tile([D + 1, 8, S, L], BF16)
            v_g = gv.tile([128, 8, S, 2, D], BF16)
            if g == 0:
                # startup: split the first q load so E0/E1's DMA-completion
                # sems (+900ns each) arrive earlier; k stays whole so B0-B3
                # are not delayed
                nc.sync.dma_start(kt_g[:, 0:4], kte[nb, :, h0:h0 + 4])
                nc.sync.dma_start(qt_g[:, 0:2], qte[nb, :, h0:h0 + 2])
                nc.sync.dma_start(qt_g[:, 2:4], qte[nb, :, h0 + 2:h0 + 4])
                nc.sync.dma_start(v_g[:, 0:4], vh[nb, :, h0:h0 + 4])
                for hh in (4,):
                    nc.sync.dma_start(kt_g[:, hh:hh + 4], kte[nb, :, h0 + hh:h0 + hh + 4])
                    nc.sync.dma_start(qt_g[:, hh:hh + 4], qte[nb, :, h0 + hh:h0 + hh + 4])
                    nc.sync.dma_start(v_g[:, hh:hh + 4], vh[nb, :, h0 + hh:h0 + hh + 4])
                return (kt_g, qt_g, v_g)
            for hh in (0, 4):
                nc.sync.dma_start(kt_g[:, hh:hh + 4], kte[nb, :, h0 + hh:h0 + hh + 4])
                nc.sync.dma_start(qt_g[:, hh:hh + 4], qte[nb, :, h0 + hh:h0 + hh + 4])
                nc.sync.dma_start(v_g[:, hh:hh + 4], vh[nb, :, h0 + hh:h0 + hh + 4])
            return (kt_g, qt_g, v_g)

        # ---- pipeline state, keyed by unit index u = 8*g + j ----
        gtiles = {}      # g -> (kt, qt, v)
        b_ps = {}        # u -> B psum tile
        b_sb = {}        # u -> B sbuf tile
        e_ps = {}        # u -> merged energy psum tile [128, c, s, 256]
        att = {}         # u -> merged att sbuf tile
        rr = {}          # u -> merged reciprocal tile [128, c, s]
        vs = {}          # u -> {(s, c) -> vs tile}
        tmp_ps = {}      # pair -> tmp psum tile [128, 2, 256] (2 heads)
        at_t = {}        # pair -> attention-out sbuf tile [128, 2, 256]
        fin = {}         # lcfc of current drain group -> psum tile
        orow = {}        # lc -> [128, E] output row tile (both fc halves)

        # F-part order per drain slot jj: (lcfc, half); half 1 closes the
        # accumulation -> bias add + store.
        FSEQ = [(0, 0), (1, 0), (0, 1), (1, 1), (2, 0), (3, 0), (2, 1), (3, 1)]

        done_t = set()
        for u in range(NU + 5):
            g, jj = divmod(u, 8)

            # ---- prefetch ----
            if u == 0:
                nc.sync.dma_start(ast_sb[:], ast[:])
                gtiles[0] = load_group(0)
            if jj == 4 and g + 1 < NG and u < NU:
                gtiles[g + 1] = load_group(g + 1)
            if u == 1:
                nc.sync.dma_start(wo_sb[:], wo3t[:])
                nc.sync.dma_start(bo_sb[:], _bcast_rows(bo[:], 128))

            if u < NU:
                kt_g, qt_g, v_g = gtiles[g]

                # ---- B_u = Ast^T @ kTe : [65, s, 256] (PE) ----
                # fill-only: B1 borrows an eps slot (eps idle until E_0) so
                # it need not wait for bcopy_0 to free the single bps bank
                if u == 1:
                    b_ps[u] = eps.tile([D + 1, S, L], F32, name="e_ps")
                elif u in (2, 3):
                    # fps is likewise idle until the first projection (slot 8)
                    b_ps[u] = fps.tile([D + 1, S, L], F32, name="fin")
                else:
                    b_ps[u] = bps.tile([D + 1, S, L], F32, name="b_ps")
                nc.tensor.matmul(b_ps[u][:], ast_sb[:], kt_g[:, jj, :, :],
                                 start=True, stop=True)
                # ---- B copy psum->sbuf (Act / DVE alternating) ----
                b_sb[u] = bsb.tile([D + 1, S, L], BF16, name="b_sb")
                if u < 2:
                    # fill: copy per s-half so the energy matmuls can begin
                    # as soon as their stationary half lands
                    for s in range(S):
                        nc.vector.tensor_copy(b_sb[u][:, s], b_ps[u][:, s])
                elif jj % 2 == 1:
                    nc.scalar.copy(b_sb[u][:], b_ps[u][:])
                else:
                    nc.vector.tensor_copy(b_sb[u][:], b_ps[u][:])

            # ---- attnxv: tmp[(s,d'), jpair, q] in psum (PE) ----
            # (issued before the energy block so the attn-out copy lands
            # early in Act's queue, ahead of exp)
            uts = [u - 3]
            if u == NU:
                uts = [NU - 3, NU - 2]
            elif u == NU + 1:
                uts = [NU - 1]
            for ut in uts:
                if not (0 <= ut < NU) or ut in done_t:
                    continue
                done_t.add(ut)
                pair = ut - (ut % 2)
                half = ut % 2
                if half == 0:
                    tmp_ps[pair] = tps.tile([128, 2, L], F32, name="tmp_ps")
                nmm = 0
                for s in range(S):
                    for c in range(2):
                        nc.tensor.matmul(
                            tmp_ps[pair][:, half, :],
                            vs[ut][(s, c)][:],
                            att[ut][:, c, s, :],
                            start=(nmm == 0), stop=(nmm == 3))
                        nmm += 1
                if half == 1:
                    at_t[pair] = atp.tile([128, 2, L], BF16, name="at_t")
                    nc.scalar.copy(at_t[pair][:], tmp_ps[pair][:])
                    del tmp_ps[pair]
                del vs[ut], att[ut], rr[ut], e_ps[ut]

            # ---- energy_(u-1): e^T[k, c, s, q] in one 2-bank psum (PE) ----
            up = u - 1
            if 0 <= up < NU:
                gp_, jp_ = divmod(up, 8)
                kt_p, qt_p, v_p = gtiles[gp_]
                ep = eps.tile([128, 2, S, L], F32, name="e_ps")
                for c in range(2):
                    for s in range(S):
                        nc.tensor.matmul(
                            ep[:, c, s, :],
                            b_sb[up][:, s, bass.ts(c, 128)],
                            qt_p[:, jp_, s, :],
                            start=True, stop=True)
                e_ps[up] = ep
                # ---- exp (Act), q-sum + recip (DVE), v-scale (Pool) ----
                att_u = attp.tile([128, 2, S, L], BF16, name="att_u")
                if up < 2:
                    # fill: per-c exps start as soon as each c-half of the
                    # energy psum is written (range-based deps)
                    for c in range(2):
                        nc.scalar.activation(att_u[:, c], ep[:, c],
                                             mybir.ActivationFunctionType.Exp,
                                             scale=SCALE)
                else:
                    nc.scalar.activation(att_u[:], ep[:],
                                         mybir.ActivationFunctionType.Exp,
                                         scale=SCALE)
                att[up] = att_u
                h_u = hp.tile([128, 2, S, L // 2], BF16, name="h_u")
                ss_u = ssp.tile([128, 2, S], F32, name="ss_u")
                rr_u = rrp.tile([128, 2, S], F32, name="rr_u")
                c_chunks = ([0], [1]) if up < 2 else ([0, 1],)
                vs[up] = {}
                for cs in c_chunks:
                    c0, c1 = cs[0], cs[-1] + 1
                    nc.vector.tensor_tensor(
                        h_u[:, c0:c1], att_u[:, c0:c1, :, 0:L // 2],
                        att_u[:, c0:c1, :, L // 2:L], op=mybir.AluOpType.add)
                    nc.vector.reduce_sum(ss_u[:, c0:c1], h_u[:, c0:c1],
                                         axis=mybir.AxisListType.X)
                    nc.vector.reciprocal(rr_u[:, c0:c1], ss_u[:, c0:c1])
                    for s in range(S):
                        pool, tg = (vsp0, "vs0") if s == 0 else (vsp1, "vs1")
                        for c in cs:
                            vs_sc = pool.tile([128, 2 * D], BF16, tag=tg)
                            nc.gpsimd.tensor_scalar_mul(
                                vs_sc[:, bass.ts(s, 64)], v_p[:, jp_, s, c, :],
                                rr_u[:, c, s:s + 1])
                            vs[up][(s, c)] = vs_sc
                rr[up] = rr_u
                del b_sb[up], b_ps[up]

            # ---- output projection of the previous group (PE) ----
            gd = g - 1
            if gd >= 0:
                if g == NG:
                    parts = [] if jj == 0 else FSEQ[2 * (jj - 1):2 * jj]
                else:
                    parts = FSEQ[jj:jj + 1]
                for lcfc, half in parts:
                    lc, fc = divmod(lcfc, 2)
                    if half == 0:
                        fin[lcfc] = fps.tile([128, 512], F32, name="fin")
                    for j8 in range(4 * half, 4 * half + 4):
                        nc.tensor.matmul(
                            fin[lcfc][:],
                            at_t[8 * gd + 2 * (j8 // 2)][:, j8 % 2, bass.ts(lc, 128)],
                            wo_sb[:, j8, bass.ts(fc, 512)],
                            start=(j8 == 0), stop=(j8 == 7))
                    if half == 1:
                        nbd, hgd = divmod(gd, S)
                        if g == NG:
                            # tail: bias+store each half immediately so the
                            # last serial chain ends with a half-row transfer
                            o_sb = osb.tile([128, 512], BF16, name="o_sb")
                            nc.vector.tensor_tensor(
                                o_sb[:], fin[lcfc][:],
                                bo_sb[:, bass.ts(fc, 512)],
                                op=mybir.AluOpType.add)
                            nc.sync.dma_start(
                                out[nbd, bass.ts(lc, 128), hgd, bass.ts(fc, 512)],
                                o_sb[:])
                        else:
                            # steady: one [128, 1024] row tile per lc; store
                            # once both fc halves are bias-added (contiguous)
                            if fc == 0:
                                orow[lc] = osb.tile([128, E], BF16, name="orow")
                            nc.vector.tensor_tensor(
                                orow[lc][:, bass.ts(fc, 512)], fin[lcfc][:],
                                bo_sb[:, bass.ts(fc, 512)],
                                op=mybir.AluOpType.add)
                            if fc == 1:
                                nc.sync.dma_start(
                                    out[nbd, bass.ts(lc, 128), hgd, :],
                                    orow[lc][:])
                                del orow[lc]
                        del fin[lcfc]
                        if lcfc == 3:
                            for p8 in range(0, 8, 2):
                                del at_t[8 * gd + p8]

    if fix_waits:
        fix_multi_waits(nc)
    return nc


def prep_inputs(values, keys, query, mask, Wv, Wk, Wq, Wo, bo):
    """Host-side sharding + layout prep (weight algebra + transposes only)."""
    import ml_dtypes
    bf16 = ml_dtypes.bfloat16

    q5 = query.reshape(N, L, S, H, D)
    k5 = keys.reshape(N, L, S, H, D)
    v5 = values.reshape(N, L, S, H, D)

    m = mask.reshape(N, S, E, L)[:, :, 0, :]
    m2 = np.tile(m[..., None], (1, 1, 1, H)).reshape(N, H, S, L)
    mrow = np.where(m2 == 0, np.float32(-1e20), np.float32(0.0))

    qTe = np.empty((N, D + 1, H, S, L), np.float32)
    qTe[:, :D] = q5.transpose(0, 4, 3, 2, 1)
    qTe[:, D] = mrow
    kTe = np.empty((N, D + 1, H, S, L), np.float32)
    kTe[:, :D] = k5.transpose(0, 4, 3, 2, 1)
    kTe[:, D] = 1.0
    vhh = np.ascontiguousarray(
        v5.reshape(N, 2, 128, S, H, D).transpose(0, 2, 4, 3, 1, 5))

    A = Wq.T @ Wk
    Ast = np.zeros((D + 1, D + 1), np.float32)
    Ast[:D, :D] = A.T
    Ast[D, D] = 1.0

    cp = np.arange(E)
    e_of = (cp // 128) * 128 + 2 * (cp % 64) + ((cp % 128) // 64)
    Wo2 = Wo[:, e_of]
    Wo3 = np.empty_like(Wo2)
    for blk in range(16):
        sl = slice(blk * 64, (blk + 1) * 64)
        Wo3[:, sl] = Wo2[:, sl] @ Wv
    Wo3T = np.ascontiguousarray(
        Wo3.T.reshape(8, 128, E).transpose(1, 0, 2))
    bo2 = np.ascontiguousarray(bo.reshape(1, E))

    qTe = qTe.astype(bf16)
    kTe = kTe.astype(bf16)
    vhh = vhh.astype(bf16)
    Astb = Ast.astype(bf16)
    Wo3Tb = Wo3T.astype(bf16)

    in_maps = []
    for c in range(NC):
        nsl = slice(c * NB, (c + 1) * NB)
        in_maps.append({
            "qte": np.ascontiguousarray(qTe[nsl]),
            "kte": np.ascontiguousarray(kTe[nsl]),
            "vh": np.ascontiguousarray(vhh[nsl]),
            "ast": Astb,
            "wo3t": Wo3Tb,
            "bo": bo2,
        })
    return in_maps


_NC_CACHE = []


def get_program():
    if not _NC_CACHE:
        _NC_CACHE.append(build_program())
    return _NC_CACHE[0]


def kernel(values, keys, query, mask, Wv, Wk, Wq, Wo, bo):
    values = np.asarray(values, np.float32)
    keys = np.asarray(keys, np.float32)
    query = np.asarray(query, np.float32)
    mask = np.asarray(mask)
    in_maps = prep_inputs(values, keys, query, mask,
                          np.asarray(Wv, np.float32), np.asarray(Wk, np.float32),
                          np.asarray(Wq, np.float32), np.asarray(Wo, np.float32),
                          np.asarray(bo, np.float32))
    nc = get_program()
    res = run_bass_kernel_spmd(nc, in_maps, core_ids=list(range(NC)))
    full = np.empty((N, L, S, E), np.float32)
    for c in range(NC):
        full[c * NB:(c + 1) * NB] = np.asarray(res.results[c]["out"],
                                               dtype=np.float32)
    return full


if __name__ == "__main__":
    import importlib.util
    spec = importlib.util.spec_from_file_location(
        "reference", "/root/problem/reference.py")
    ref = importlib.util.module_from_spec(spec)
    spec.loader.exec_module(ref)
    inputs = {k: np.asarray(v) for k, v in ref.setup_inputs().items()}
    got = kernel(**inputs)
    exp = np.load("/root/problem/work/ref_out.npy")
    num = np.linalg.norm((got - exp).ravel())
    den = np.linalg.norm(exp.ravel())
    print("L2 rel err:", num / den)
    print("absmax err:", np.abs(got - exp).max())
